# revision 12
# baseline (speedup 1.0000x reference)
"""Trainium2 Bass kernel for BasicTransformer2D (2-layer, C=256, NH=8, S=4096, cross LC=77).

Strategy: sequence-parallel over 8 NeuronCores (512 tokens/core), channels-first
[C, S_local] activation layout. Per layer: cross-attn + LN fully local; one
AllGather of the LN'd hidden states feeds self-attention K/V (computed
redundantly per core); FFN local. Rel-pos softmax bias is applied
multiplicatively after exp() using host-precomputed exp(bias) Toeplitz strips.
Softmax denominators come free from a ones-column appended to V; division is
exp(-ln(x)) on the scalar engine (keeps a single activation table set).
"""

import numpy as np
import ml_dtypes

import concourse.bacc as bacc
import concourse.mybir as mybir
import concourse.tile as tile
from concourse.bass_utils import run_bass_kernel_spmd

# ---- problem dims (hardcoded per spec) ----
L = 2
C = 256
NH = 8
HD = 32
B = 1
W_SP = 64
H_SP = 64
S = 4096
LC = 77
DC = 768
FF = 1024
EPS = 1e-5
N_CORES = 8
TL = S // N_CORES          # 512 tokens per core
NKT = S // 128             # 32 key tiles
GK = 2                     # key tiles per psum/exp group
NG = NKT // GK             # 16 groups
STRIP_W = 3968 + TL        # 4480 strip columns
SCALE = HD ** -0.5

F32 = mybir.dt.float32
BF16 = mybir.dt.bfloat16
AF = mybir.ActivationFunctionType
ALU = mybir.AluOpType

# ---- weight image column offsets (bf16 [128, WIMG_COLS] per layer) ----
OFF_CAQ = 0                      # ca_wq lhsT [ic2][oc2] -> 512
OFF_CAK = 512                    # ca_wk lhsT [ic6][oc2] -> 1536
OFF_CAV = 2048                   # ca_wv rhs  [ic6][256] -> 1536
OFF_CAO = 3584                   # ca_wo lhsT [ic2][oc2] -> 512
OFF_SAQ = 4096
OFF_SAK = 4608
OFF_SAV = 5120                   # sa_wv rhs [ic2][256] -> 512
OFF_SAO = 5632
OFF_W1 = 6144                    # ff_w1 lhsT [ic2][hc8] -> 2048
OFF_W2 = 8192                    # ff_w2 lhsT [hc8][oc2] -> 2048
WIMG_COLS = 10240


def _pack_lhsT(img, base, w, n_ic, n_oc):
    for ic in range(n_ic):
        for oc in range(n_oc):
            img[:, base + (ic * n_oc + oc) * 128:base + (ic * n_oc + oc + 1) * 128] = \
                w[128 * ic:128 * (ic + 1), 128 * oc:128 * (oc + 1)]


def _pack_rhs(img, base, w, n_ic, width):
    for ic in range(n_ic):
        img[:, base + ic * width:base + (ic + 1) * width] = w[128 * ic:128 * (ic + 1), :]


def build_in_maps(inputs):
    """Host-side packing: returns (list of per-core input dicts, any_bias flag)."""
    f = lambda k: np.asarray(inputs[k], np.float32)
    hs = f("hidden_state").reshape(C, S)                       # [C, S] channels-first
    ctx = f("context").reshape(LC, DC)
    ctxT = np.ascontiguousarray(ctx.T)                          # [768, 77]

    cn_g, cn_b = f("cn_g"), f("cn_b")
    sn_g, sn_b = f("sn_g"), f("sn_b")
    fn_g, fn_b = f("fn_g"), f("fn_b")

    wimgs, bimgs = [], []
    for l in range(L):
        img = np.zeros((128, WIMG_COLS), np.float32)
        caq = cn_g[l][:, None] * f("ca_wq")[l]
        saq = sn_g[l][:, None] * f("sa_wq")[l]
        sak = sn_g[l][:, None] * f("sa_wk")[l]
        sav = sn_g[l][:, None] * f("sa_wv")[l]
        w1 = fn_g[l][:, None] * f("ff_w1")[l]
        _pack_lhsT(img, OFF_CAQ, caq, 2, 2)
        _pack_lhsT(img, OFF_CAK, f("ca_wk")[l], 6, 2)
        _pack_rhs(img, OFF_CAV, f("ca_wv")[l], 6, 256)
        _pack_lhsT(img, OFF_CAO, f("ca_wo")[l], 2, 2)
        _pack_lhsT(img, OFF_SAQ, saq, 2, 2)
        _pack_lhsT(img, OFF_SAK, sak, 2, 2)
        _pack_rhs(img, OFF_SAV, sav, 2, 256)
        _pack_lhsT(img, OFF_SAO, f("sa_wo")[l], 2, 2)
        _pack_lhsT(img, OFF_W1, w1, 2, 8)
        _pack_lhsT(img, OFF_W2, f("ff_w2")[l], 8, 2)
        wimgs.append(img.astype(ml_dtypes.bfloat16))

        # effective biases (all zero for the spec'd inputs, but kept generic)
        bq_ca = cn_b[l] @ f("ca_wq")[l]
        bq_sa = sn_b[l] @ f("sa_wq")[l]
        bv_sa = sn_b[l] @ f("sa_wv")[l]
        b1 = fn_b[l] @ f("ff_w1")[l] + f("ff_b1")[l]
        bo_ca = f("ca_bo")[l]
        bo_sa = f("sa_bo")[l]
        b2 = f("ff_b2")[l]
        # bias image fp32 [128, 20]: bq_ca(2) bo_ca(2) bq_sa(2) bv_sa(2) bo_sa(2) b1(8) b2(2)
        bimg = np.zeros((128, 20), np.float32)
        bimg[:, 0:2] = bq_ca.reshape(2, 128).T
        bimg[:, 2:4] = bo_ca.reshape(2, 128).T
        bimg[:, 4:6] = bq_sa.reshape(2, 128).T
        bimg[:, 6:8] = bv_sa.reshape(2, 128).T
        bimg[:, 8:10] = bo_sa.reshape(2, 128).T
        bimg[:, 10:18] = b1.reshape(8, 128).T
        bimg[:, 18:20] = b2.reshape(2, 128).T
        bimgs.append(bimg)

    any_bias = any(np.abs(b).max() > 0 for b in bimgs)

    cvec = np.zeros((128, 4), np.float32)
    cvec[:, 0] = 1.0
    cvec[:, 1] = 1.0 / C
    cvec[:, 2] = EPS
    crow = np.zeros((1, 256), np.float32)
    crow[0, :128] = 1.0
    crow[0, 128:] = 1.0 / C

    sa_tab = f("sa_tab")                                        # [L, 2S-1, NH]
    ca_tab = f("ca_tab")                                        # [L, S+LC-1, NH]

    in_maps = []
    u = np.arange(STRIP_W)
    p = np.arange(128)
    qq = np.arange(TL)
    kk = np.arange(LC)
    for c in range(N_CORES):
        m = {
            "x0": np.ascontiguousarray(hs[:, c * TL:(c + 1) * TL].reshape(2, 128, TL)),
            "ctxT": ctxT.reshape(6, 128, LC).astype(ml_dtypes.bfloat16),
            "cvec": cvec,
            "crow": crow,
        }
        for l in range(L):
            m[f"wimg{l}"] = wimgs[l]
            m[f"bimg{l}"] = bimgs[l]
            # self-attn exp(bias) strips: strip[h][p, u] = exp(tab[512c + u - p + 127, h])
            idx = 512 * c + u[None, :] - p[:, None] + 127       # in [0, 8190]
            np.clip(idx, 0, 2 * S - 2, out=idx)
            strip = np.exp(sa_tab[l][idx, :])                   # [128, STRIP_W, NH]
            m[f"ebsa{l}"] = np.ascontiguousarray(
                strip.transpose(2, 0, 1)).astype(ml_dtypes.bfloat16)
            # cross-attn exp(bias): [77, NH*512]
            cidx = 512 * c + qq[None, :] - kk[:, None] + (LC - 1)
            ebc = np.exp(ca_tab[l][cidx, :])                    # [77, 512, NH]
            m[f"ebca{l}"] = np.ascontiguousarray(
                ebc.transpose(0, 2, 1).reshape(LC, NH * TL)).astype(ml_dtypes.bfloat16)
        in_maps.append(m)
    return in_maps, any_bias


def build_program(any_bias=False, stage=6, layers=L):
    """Builds the Bacc/TileContext SPMD program (one NEFF, all 8 cores).

    stage: 1=LN1 only, 2=+CA, 3=+AG/xg, 4=+KVq prep, 5=+SA, 6=full
    """
    core_ids = list(range(N_CORES))
    nc = bacc.Bacc("TRN2", target_bir_lowering=False, debug=False,
                   num_devices=N_CORES)

    x0_d = nc.dram_tensor("x0", [2, 128, TL], F32, kind="ExternalInput")
    ctxT_d = nc.dram_tensor("ctxT", [6, 128, LC], BF16, kind="ExternalInput")
    cvec_d = nc.dram_tensor("cvec", [128, 4], F32, kind="ExternalInput")
    crow_d = nc.dram_tensor("crow", [1, 256], F32, kind="ExternalInput")
    wimg_d = [nc.dram_tensor(f"wimg{l}", [128, WIMG_COLS], BF16, kind="ExternalInput") for l in range(L)]
    bimg_d = [nc.dram_tensor(f"bimg{l}", [128, 20], F32, kind="ExternalInput") for l in range(L)]
    ebsa_d = [nc.dram_tensor(f"ebsa{l}", [NH, 128, STRIP_W], BF16, kind="ExternalInput") for l in range(L)]
    ebca_d = [nc.dram_tensor(f"ebca{l}", [LC, NH * TL], BF16, kind="ExternalInput") for l in range(L)]
    y_d = nc.dram_tensor("y", [2, 128, TL], F32, kind="ExternalOutput")

    ag_in_d = [nc.dram_tensor(f"ag_in{l}", [2, 128, TL], BF16) for l in range(L)]
    ag_out_d = [nc.dram_tensor(f"ag_out{l}", [N_CORES, 2, 128, TL], BF16,
                               addr_space="Shared") for l in range(L)]

    with tile.TileContext(nc) as tc:
        with (
            tc.tile_pool(name="pconst", bufs=1) as pconst,
            tc.tile_pool(name="ppers", bufs=1) as ppers,
            tc.tile_pool(name="px", bufs=2) as px,
            tc.tile_pool(name="pstrip", bufs=2) as pstrip,
            tc.tile_pool(name="pwork", bufs=3) as pwork,
            tc.tile_pool(name="prow", bufs=6) as prow,
            tc.tile_pool(name="pca_p", bufs=3) as pca_p,
            tc.tile_pool(name="psc", bufs=2, space="PSUM") as psc,
            tc.tile_pool(name="pacc", bufs=2, space="PSUM") as pacc,
            tc.tile_pool(name="ppp", bufs=2, space="PSUM") as ppp,
        ):
            # ---- constants / weights (x + layer-0 weights first; layer-1 deferred) ----
            x_t = []
            for ci in range(2):
                xt = px.tile([128, TL], F32, tag=f"x{ci}")
                nc.sync.dma_start(out=xt[:], in_=x0_d[ci])
                x_t.append(xt)
            cvec_t = pconst.tile([128, 4], F32, tag="cvec")
            crow_t = pconst.tile([1, 256], F32, tag="crow")
            ctx_t = pconst.tile([128, 6 * LC], BF16, tag="ctx")
            nc.sync.dma_start(out=cvec_t[:], in_=cvec_d[:])
            nc.sync.dma_start(out=crow_t[:], in_=crow_d[:])
            nc.sync.dma_start(
                out=ctx_t[:].rearrange("p (c t) -> p c t", c=6),
                in_=ctxT_d[:].rearrange("c p t -> p c t"))
            wimg_t, bimg_t = [], []
            for l in range(L):
                wt = pconst.tile([128, WIMG_COLS], BF16, tag=f"wimg{l}", name="wt")
                wimg_t.append(wt)
                bt = pconst.tile([128, 20], F32, tag=f"bimg{l}", name="bt")
                bimg_t.append(bt)
            nc.sync.dma_start(out=wimg_t[0][:], in_=wimg_d[0][:])
            nc.sync.dma_start(out=bimg_t[0][:], in_=bimg_d[0][:])

            ones_col = cvec_t[:, 0:1]
            ones_row = crow_t[0:1, 0:128]

            def w_ap(l, base, i, j, n_j, width=128):
                off = base + (i * n_j + j) * width
                return wimg_t[l][:, off:off + width]

            def layernorm(xin):
                ps_sum = ppp.tile([1, TL], F32, tag="pp")
                nc.tensor.matmul(ps_sum[:], ones_col, xin[0][:], start=True, stop=False)
                nc.tensor.matmul(ps_sum[:], ones_col, xin[1][:], start=False, stop=True)
                sq = []
                for ci in range(2):
                    st = pwork.tile([128, TL], F32, tag="sq")
                    nc.vector.tensor_mul(st[:], xin[ci][:], xin[ci][:])
                    sq.append(st)
                ps_sq = ppp.tile([1, TL], F32, tag="pp")
                nc.tensor.matmul(ps_sq[:], ones_col, sq[0][:], start=True, stop=False)
                nc.tensor.matmul(ps_sq[:], ones_col, sq[1][:], start=False, stop=True)
                m_sb = prow.tile([1, TL], F32, tag="row")
                nc.scalar.activation(m_sb[:], ps_sum[:], AF.Copy, scale=1.0 / C)
                m2_sb = prow.tile([1, TL], F32, tag="row")
                nc.scalar.activation(m2_sb[:], ps_sum[:], AF.Square, scale=1.0 / C)
                var_sb = prow.tile([1, TL], F32, tag="row")
                nc.vector.scalar_tensor_tensor(var_sb[:], ps_sq[:], 1.0 / C, m2_sb[:],
                                               op0=ALU.mult, op1=ALU.subtract)
                lv_sb = prow.tile([1, TL], F32, tag="row")
                nc.scalar.activation(lv_sb[:], var_sb[:], AF.Ln, bias=cvec_t[0:1, 2:3])
                rstd_sb = prow.tile([1, TL], F32, tag="row")
                nc.scalar.activation(rstd_sb[:], lv_sb[:], AF.Exp, scale=-0.5)
                ps_mbc = ppp.tile([128, TL], F32, tag="pp")
                nc.tensor.matmul(ps_mbc[:], ones_row, m_sb[:], start=True, stop=True)
                ps_rbc = ppp.tile([128, TL], F32, tag="pp")
                nc.tensor.matmul(ps_rbc[:], ones_row, rstd_sb[:], start=True, stop=True)
                out = []
                for ci in range(2):
                    tt = pwork.tile([128, TL], F32, tag="lnt")
                    nc.vector.tensor_sub(tt[:], xin[ci][:], ps_mbc[:])
                    ot = pwork.tile([128, TL], BF16, tag=f"xs{ci}")
                    nc.vector.tensor_mul(ot[:], tt[:], ps_rbc[:])
                    out.append(ot)
                return out

            def norm_head_out(l, o_ps, o_norm, h, bias_col):
                """o_ps: psum [33, TL] (row0 = sums); writes o_norm chunk rows."""
                lr = prow.tile([1, TL], F32, tag="row")
                nc.scalar.activation(lr[:], o_ps[32:33, :], AF.Ln)
                rr = prow.tile([1, TL], F32, tag="row")
                nc.scalar.activation(rr[:], lr[:], AF.Exp, scale=-1.0)
                rb_ps = ppp.tile([32, TL], F32, tag="pp")
                nc.tensor.matmul(rb_ps[:], crow_t[0:1, 0:32], rr[:], start=True, stop=True)
                rb_sb = pwork.tile([32, TL], F32, tag="rb")
                nc.vector.tensor_copy(rb_sb[:], rb_ps[:])
                r0 = 32 * (h % 4)
                dst = o_norm[h // 4][r0:r0 + 32, :]
                nc.vector.tensor_mul(dst, o_ps[0:32, :], rb_sb[:])
                if any_bias and bias_col is not None:
                    nc.vector.tensor_scalar_add(
                        dst, dst, bimg_t[l][r0:r0 + 32, bias_col + h // 4:bias_col + h // 4 + 1])

            def proj_residual(l, base_off, o_norm, bo_col):
                newx = []
                for oc in range(2):
                    ps = ppp.tile([128, TL], F32, tag="pp")
                    for ic in range(2):
                        nc.tensor.matmul(ps[:], w_ap(l, base_off, ic, oc, 2),
                                         o_norm[ic][:], start=(ic == 0), stop=(ic == 1))
                    nx = px.tile([128, TL], F32, tag=f"x{oc}")
                    nc.vector.scalar_tensor_tensor(
                        nx[:], ps[:], bimg_t[l][:, bo_col + oc:bo_col + oc + 1],
                        x_t[oc][:], op0=ALU.add, op1=ALU.add)
                    newx.append(nx)
                return newx

            for l in range(layers):
                # ================= cross-attention =================
                xs1 = layernorm(x_t)
                if stage < 2:
                    x_t = [xs1[0], xs1[1]]  # dummy: emit LN result
                    break
                ebca_t = pstrip.tile([LC, NH * TL], BF16, tag="ebca", bufs=1)
                nc.sync.dma_start(out=ebca_t[:], in_=ebca_d[l][:])
                q_ca = []
                for oc in range(2):
                    ps = ppp.tile([128, TL], F32, tag="pp")
                    for ic in range(2):
                        nc.tensor.matmul(ps[:], w_ap(l, OFF_CAQ, ic, oc, 2),
                                         xs1[ic][:], start=(ic == 0), stop=(ic == 1))
                    qt = ppers.tile([128, TL], BF16, tag=f"qca{oc}")
                    if any_bias:
                        nc.vector.tensor_scalar_add(qt[:], ps[:], bimg_t[l][:, oc:oc + 1])
                    else:
                        nc.vector.tensor_copy(qt[:], ps[:])
                    q_ca.append(qt)
                kca = []
                for oc in range(2):
                    ps = ppp.tile([128, LC], F32, tag="pp")
                    for ic in range(6):
                        nc.tensor.matmul(ps[:], w_ap(l, OFF_CAK, ic, oc, 2),
                                         ctx_t[:, ic * LC:(ic + 1) * LC],
                                         start=(ic == 0), stop=(ic == 5))
                    kt = ppers.tile([128, LC], BF16, tag=f"kca{oc}")
                    nc.vector.tensor_copy(kt[:], ps[:])
                    kca.append(kt)
                ps_v = ppp.tile([LC, 256], F32, tag="pp")
                for ic in range(6):
                    nc.tensor.matmul(
                        ps_v[:], ctx_t[:, ic * LC:(ic + 1) * LC],
                        wimg_t[l][:, OFF_CAV + ic * 256:OFF_CAV + (ic + 1) * 256],
                        start=(ic == 0), stop=(ic == 5))
                vca = ppers.tile([LC, NH * 33], BF16, tag="vca")
                vca_r = vca[:].rearrange("p (h c) -> p h c", h=NH)
                nc.vector.memset(vca_r[:, :, 32:33], 1.0)
                nc.vector.tensor_copy(
                    vca_r[:, :, 0:32], ps_v[:].rearrange("p (h c) -> p h c", h=NH))
                o_norm = [ppers.tile([128, TL], BF16, tag=f"onorm{ci}", name=f"onorm{ci}") for ci in range(2)]
                for hp in range(NH // 2):          # head pairs through the wide sc tiles
                    ps_s = psc.tile([128, 2 * TL], F32, tag="sc", name="ps_s")
                    for b in range(2):
                        h = 2 * hp + b
                        r0 = 32 * (h % 4)
                        k_ap = kca[h // 4][r0:r0 + 32, :]
                        q_ap = q_ca[h // 4][r0:r0 + 32, :]
                        if h % 4 == 3:  # PE can't read base partition 96 (quadrant-3 bug)
                            ks = pwork.tile([32, LC], BF16, tag="kscr_ca", name="ks", bufs=1)
                            nc.vector.tensor_copy(ks[:], k_ap)
                            qs = pwork.tile([32, TL], BF16, tag="qscr_ca", name="qs", bufs=1)
                            nc.vector.tensor_copy(qs[:], q_ap)
                            k_ap, q_ap = ks[:], qs[:]
                        nc.tensor.matmul(ps_s[0:LC, TL * b:TL * (b + 1)], k_ap, q_ap,
                                         start=True, stop=True)
                    pc = pca_p.tile([LC, 2 * TL], BF16, tag="pca")
                    nc.scalar.activation(pc[:], ps_s[0:LC, :], AF.Exp, scale=SCALE)
                    pc2 = pca_p.tile([LC, 2 * TL], BF16, tag="pca2")
                    nc.vector.tensor_mul(pc2[:], pc[:],
                                         ebca_t[:, 2 * hp * TL:2 * (hp + 1) * TL])
                    for b in range(2):
                        h = 2 * hp + b
                        o_ps = pacc.tile([33, TL], F32, tag="acc")
                        nc.tensor.matmul(o_ps[:], vca[:, h * 33:(h + 1) * 33],
                                         pc2[:, TL * b:TL * (b + 1)],
                                         start=True, stop=True)
                        norm_head_out(l, o_ps, o_norm, h, None)
                x_t = proj_residual(l, OFF_CAO, o_norm, 2)
                if stage < 3:
                    break

                # ================= self-attention =================
                xs2 = layernorm(x_t)
                for ci in range(2):
                    nc.sync.dma_start(out=ag_in_d[l][ci], in_=xs2[ci][:])
                nc.gpsimd.collective_compute(
                    "AllGather", ALU.bypass,
                    replica_groups=[core_ids],
                    ins=[ag_in_d[l][:]],
                    outs=[ag_out_d[l][:]],
                )
                if l == 0:
                    nc.sync.dma_start(out=wimg_t[1][:], in_=wimg_d[1][:])
                    nc.sync.dma_start(out=bimg_t[1][:], in_=bimg_d[1][:])
                xg = []
                for ci in range(2):
                    xgt = ppers.tile([128, S], BF16, tag=f"xg{ci}")
                    nc.sync.dma_start(
                        out=xgt[:].rearrange("p (s t) -> p s t", s=N_CORES),
                        in_=ag_out_d[l][:, ci].rearrange("s p t -> p s t"))
                    xg.append(xgt)
                if stage < 4:
                    x_t = [xs2[0], xs2[1]]
                    break
                qT = []
                for oc in range(2):
                    ps = ppp.tile([128, TL], F32, tag="pp")
                    for ic in range(2):
                        nc.tensor.matmul(ps[:], w_ap(l, OFF_SAQ, ic, oc, 2),
                                         xs2[ic][:], start=(ic == 0), stop=(ic == 1))
                    qt = ppers.tile([128, TL], BF16, tag=f"qsa{oc}")
                    if any_bias:
                        nc.vector.tensor_scalar_add(qt[:], ps[:], bimg_t[l][:, 4 + oc:5 + oc])
                    else:
                        nc.vector.tensor_copy(qt[:], ps[:])
                    qT.append(qt)
                kT = []
                for oc in range(2):
                    kt = ppers.tile([128, S], BF16, tag=f"ksa{oc}")
                    for w in range(8):
                        ps = ppp.tile([128, 512], F32, tag="pp")
                        for ic in range(2):
                            nc.tensor.matmul(ps[:], w_ap(l, OFF_SAK, ic, oc, 2),
                                             xg[ic][:, 512 * w:512 * (w + 1)],
                                             start=(ic == 0), stop=(ic == 1))
                        nc.vector.tensor_copy(kt[:, 512 * w:512 * (w + 1)], ps[:])
                    kT.append(kt)
                vaug = ppers.tile([128, NKT * 264], BF16, tag="vaug")
                vaug_r = vaug[:].rearrange("p (k h c) -> p k h c", k=NKT, h=NH)
                nc.vector.memset(vaug_r[:, :, :, 32:33], 1.0)
                for kt_i in range(NKT):
                    ps = ppp.tile([128, 256], F32, tag="pp")
                    for ic in range(2):
                        nc.tensor.matmul(
                            ps[:], xg[ic][:, 128 * kt_i:128 * (kt_i + 1)],
                            wimg_t[l][:, OFF_SAV + ic * 256:OFF_SAV + (ic + 1) * 256],
                            start=(ic == 0), stop=(ic == 1))
                    nc.vector.tensor_copy(
                        vaug_r[:, kt_i, :, 0:32],
                        ps[:].rearrange("p (h c) -> p h c", h=NH))
                if stage < 5:
                    break
                o_norm = [ppers.tile([128, TL], BF16, tag=f"onorm{ci}", name=f"onorm{ci}") for ci in range(2)]
                for h in range(NH):
                    strip = pstrip.tile([128, STRIP_W], BF16, tag="strip")
                    nc.sync.dma_start(out=strip[:], in_=ebsa_d[l][h])
                    o_ps = pacc.tile([33, TL], F32, tag="acc")
                    r0 = 32 * (h % 4)
                    if h % 4 == 3:  # PE can't read base partition 96 (quadrant-3 bug)
                        khs = pwork.tile([32, S], BF16, tag="kscr_sa", name="khs", bufs=1)
                        nc.vector.tensor_copy(khs[:], kT[h // 4][r0:r0 + 32, :])
                        qhs = pwork.tile([32, TL], BF16, tag="qscr_sa", name="qhs", bufs=1)
                        nc.vector.tensor_copy(qhs[:], qT[h // 4][r0:r0 + 32, :])
                        kh, qh, r0 = khs, qhs[:], 0
                    else:
                        kh = kT[h // 4]
                        qh = qT[h // 4][r0:r0 + 32, :]
                    for g in range(NG):
                        ps_s = psc.tile([128, GK * TL], F32, tag="sc")
                        for j in range(GK):
                            t = g * GK + j
                            nc.tensor.matmul(
                                ps_s[:, TL * j:TL * (j + 1)],
                                kh[r0:r0 + 32, 128 * t:128 * (t + 1)],
                                qh, start=True, stop=True)
                        pt = pwork.tile([128, GK * TL], BF16, tag="pt")
                        nc.scalar.activation(pt[:], ps_s[:], AF.Exp, scale=SCALE)
                        p2 = pwork.tile([128, GK * TL], BF16, tag="p2")
                        for j in range(GK):
                            t = g * GK + j
                            off = 3968 - 128 * t
                            nc.vector.tensor_mul(
                                p2[:, TL * j:TL * (j + 1)],
                                pt[:, TL * j:TL * (j + 1)],
                                strip[:, off:off + TL])
                        for j in range(GK):
                            t = g * GK + j
                            nc.tensor.matmul(
                                o_ps[:], vaug[:, 264 * t + 33 * h:264 * t + 33 * (h + 1)],
                                p2[:, TL * j:TL * (j + 1)],
                                start=(t == 0), stop=(t == NKT - 1))
                    norm_head_out(l, o_ps, o_norm, h, 6)
                x_t = proj_residual(l, OFF_SAO, o_norm, 8)
                if stage < 6:
                    break

                # ================= FFN (replaces x) =================
                xs3 = layernorm(x_t)
                ps_f = [ppp.tile([128, TL], F32, tag="pp", name="ps_f") for _ in range(2)]
                for hp in range(4):                 # hidden-chunk pairs via sc tiles
                    ps_h = psc.tile([128, 2 * TL], F32, tag="sc", name="ps_h")
                    for b in range(2):
                        hc = 2 * hp + b
                        for ic in range(2):
                            nc.tensor.matmul(ps_h[:, TL * b:TL * (b + 1)],
                                             w_ap(l, OFF_W1, ic, hc, 8),
                                             xs3[ic][:], start=(ic == 0), stop=(ic == 1))
                    gt = pwork.tile([128, 2 * TL], BF16, tag="gelu", name="gt")
                    if any_bias:
                        for b in range(2):
                            nc.scalar.activation(
                                gt[:, TL * b:TL * (b + 1)], ps_h[:, TL * b:TL * (b + 1)],
                                AF.Gelu, bias=bimg_t[l][:, 10 + 2 * hp + b:11 + 2 * hp + b])
                    else:
                        nc.scalar.activation(gt[:], ps_h[:], AF.Gelu)
                    for b in range(2):
                        hc = 2 * hp + b
                        for oc in range(2):
                            nc.tensor.matmul(ps_f[oc][:], w_ap(l, OFF_W2, hc, oc, 2),
                                             gt[:, TL * b:TL * (b + 1)],
                                             start=(hc == 0), stop=(hc == 7))
                newx = []
                for oc in range(2):
                    nx = px.tile([128, TL], F32, tag=f"x{oc}")
                    nc.vector.tensor_scalar_add(nx[:], ps_f[oc][:],
                                                bimg_t[l][:, 18 + oc:19 + oc])
                    newx.append(nx)
                x_t = newx

            for ci in range(2):
                if x_t[ci].dtype != F32:
                    cast = ppers.tile([128, TL], F32, tag=f"cast{ci}", name="cast")
                    nc.vector.tensor_copy(cast[:], x_t[ci][:])
                    nc.sync.dma_start(out=y_d[ci], in_=cast[:])
                else:
                    nc.sync.dma_start(out=y_d[ci], in_=x_t[ci][:])

    nc.compile()
    return nc


_CACHE = {}


def _get_program(any_bias):
    key = bool(any_bias)
    if key not in _CACHE:
        _CACHE[key] = build_program(any_bias=key)
    return _CACHE[key]


def kernel(**inputs) -> np.ndarray:
    in_maps, any_bias = build_in_maps(inputs)
    nc = _get_program(any_bias)
    res = run_bass_kernel_spmd(nc, in_maps, list(range(N_CORES)))
    slices = [np.asarray(res.results[c]["y"], np.float32).reshape(C, TL)
              for c in range(N_CORES)]
    full = np.concatenate(slices, axis=1)                      # [C, S]
    return full.reshape(B, C, W_SP, H_SP).astype(np.float32)


# revision 13
# speedup vs baseline: 8.7021x; 8.7021x over previous
"""Trainium2 Bass kernel for BasicTransformer2D (2-layer, C=256, NH=8, S=4096, cross LC=77).

Strategy: sequence-parallel over 8 NeuronCores (512 tokens/core), channels-first
[C, S_local] activation layout. Per layer: cross-attn + LN fully local; one
AllGather of the LN'd hidden states feeds self-attention K/V (computed
redundantly per core); FFN local. Rel-pos softmax bias is applied
multiplicatively after exp() using host-precomputed exp(bias) Toeplitz strips.
Softmax denominators come free from a ones-column appended to V; division is
exp(-ln(x)) on the scalar engine (keeps a single activation table set).
"""

import numpy as np
import ml_dtypes

import concourse.bacc as bacc
import concourse.mybir as mybir
import concourse.tile as tile
from concourse.bass_utils import run_bass_kernel_spmd

# ---- problem dims (hardcoded per spec) ----
L = 2
C = 256
NH = 8
HD = 32
B = 1
W_SP = 64
H_SP = 64
S = 4096
LC = 77
DC = 768
FF = 1024
EPS = 1e-5
N_CORES = 8
TL = S // N_CORES          # 512 tokens per core
NKT = S // 128             # 32 key tiles
GK = 2                     # key tiles per psum/exp group
NG = NKT // GK             # 16 groups
STRIP_W = 3968 + TL        # 4480 strip columns
SCALE = HD ** -0.5

F32 = mybir.dt.float32
BF16 = mybir.dt.bfloat16
AF = mybir.ActivationFunctionType
ALU = mybir.AluOpType

# ---- weight image column offsets (bf16 [128, WIMG_COLS] per layer) ----
OFF_CAQ = 0                      # ca_wq lhsT [ic2][oc2] -> 512
OFF_CAK = 512                    # ca_wk lhsT [ic6][oc2] -> 1536
OFF_CAV = 2048                   # ca_wv rhs  [ic6][256] -> 1536
OFF_CAO = 3584                   # ca_wo lhsT [ic2][oc2] -> 512
OFF_SAQ = 4096
OFF_SAK = 4608
OFF_SAV = 5120                   # sa_wv rhs [ic2][256] -> 512
OFF_SAO = 5632
OFF_W1 = 6144                    # ff_w1 lhsT [ic2][hc8] -> 2048
OFF_W2 = 8192                    # ff_w2 lhsT [hc8][oc2] -> 2048
WIMG_COLS = 10240


def _pack_lhsT(img, base, w, n_ic, n_oc):
    for ic in range(n_ic):
        for oc in range(n_oc):
            img[:, base + (ic * n_oc + oc) * 128:base + (ic * n_oc + oc + 1) * 128] = \
                w[128 * ic:128 * (ic + 1), 128 * oc:128 * (oc + 1)]


def _pack_rhs(img, base, w, n_ic, width):
    for ic in range(n_ic):
        img[:, base + ic * width:base + (ic + 1) * width] = w[128 * ic:128 * (ic + 1), :]


def build_in_maps(inputs):
    """Host-side packing: returns (list of per-core input dicts, any_bias flag)."""
    f = lambda k: np.asarray(inputs[k], np.float32)
    hs = f("hidden_state").reshape(C, S)                       # [C, S] channels-first
    ctx = f("context").reshape(LC, DC)
    ctxT = np.ascontiguousarray(ctx.T)                          # [768, 77]

    cn_g, cn_b = f("cn_g"), f("cn_b")
    sn_g, sn_b = f("sn_g"), f("sn_b")
    fn_g, fn_b = f("fn_g"), f("fn_b")

    wimgs, bimgs = [], []
    for l in range(L):
        img = np.zeros((128, WIMG_COLS), np.float32)
        caq = cn_g[l][:, None] * f("ca_wq")[l]
        saq = sn_g[l][:, None] * f("sa_wq")[l]
        sak = sn_g[l][:, None] * f("sa_wk")[l]
        sav = sn_g[l][:, None] * f("sa_wv")[l]
        w1 = fn_g[l][:, None] * f("ff_w1")[l]
        _pack_lhsT(img, OFF_CAQ, caq, 2, 2)
        _pack_lhsT(img, OFF_CAK, f("ca_wk")[l], 6, 2)
        _pack_rhs(img, OFF_CAV, f("ca_wv")[l], 6, 256)
        _pack_lhsT(img, OFF_CAO, f("ca_wo")[l], 2, 2)
        _pack_lhsT(img, OFF_SAQ, saq, 2, 2)
        _pack_lhsT(img, OFF_SAK, sak, 2, 2)
        _pack_rhs(img, OFF_SAV, sav, 2, 256)
        _pack_lhsT(img, OFF_SAO, f("sa_wo")[l], 2, 2)
        _pack_lhsT(img, OFF_W1, w1, 2, 8)
        _pack_lhsT(img, OFF_W2, f("ff_w2")[l], 8, 2)
        wimgs.append(img.astype(ml_dtypes.bfloat16))

        # effective biases (all zero for the spec'd inputs, but kept generic)
        bq_ca = cn_b[l] @ f("ca_wq")[l]
        bq_sa = sn_b[l] @ f("sa_wq")[l]
        bv_sa = sn_b[l] @ f("sa_wv")[l]
        b1 = fn_b[l] @ f("ff_w1")[l] + f("ff_b1")[l]
        bo_ca = f("ca_bo")[l]
        bo_sa = f("sa_bo")[l]
        b2 = f("ff_b2")[l]
        # bias image fp32 [128, 20]: bq_ca(2) bo_ca(2) bq_sa(2) bv_sa(2) bo_sa(2) b1(8) b2(2)
        bimg = np.zeros((128, 20), np.float32)
        bimg[:, 0:2] = bq_ca.reshape(2, 128).T
        bimg[:, 2:4] = bo_ca.reshape(2, 128).T
        bimg[:, 4:6] = bq_sa.reshape(2, 128).T
        bimg[:, 6:8] = bv_sa.reshape(2, 128).T
        bimg[:, 8:10] = bo_sa.reshape(2, 128).T
        bimg[:, 10:18] = b1.reshape(8, 128).T
        bimg[:, 18:20] = b2.reshape(2, 128).T
        bimgs.append(bimg)

    any_bias = any(np.abs(b).max() > 0 for b in bimgs)

    cvec = np.zeros((128, 4), np.float32)
    cvec[:, 0] = 1.0
    cvec[:, 1] = 1.0 / C
    cvec[:, 2] = EPS
    crow = np.zeros((1, 256), np.float32)
    crow[0, :128] = 1.0
    crow[0, 128:] = 1.0 / C

    sa_tab = f("sa_tab")                                        # [L, 2S-1, NH]
    ca_tab = f("ca_tab")                                        # [L, S+LC-1, NH]

    in_maps = []
    u = np.arange(STRIP_W)
    p = np.arange(128)
    qq = np.arange(TL)
    kk = np.arange(LC)
    for c in range(N_CORES):
        m = {
            "x0": np.ascontiguousarray(hs[:, c * TL:(c + 1) * TL].reshape(2, 128, TL)),
            "ctxT": ctxT.reshape(6, 128, LC).astype(ml_dtypes.bfloat16),
            "cvec": cvec,
            "crow": crow,
        }
        for l in range(L):
            m[f"wimg{l}"] = wimgs[l]
            m[f"bimg{l}"] = bimgs[l]
            # self-attn exp(bias) strips: strip[h][p, u] = exp(tab[512c + u - p + 127, h])
            idx = 512 * c + u[None, :] - p[:, None] + 127       # in [0, 8190]
            np.clip(idx, 0, 2 * S - 2, out=idx)
            strip = np.exp(sa_tab[l][idx, :])                   # [128, STRIP_W, NH]
            m[f"ebsa{l}"] = np.ascontiguousarray(
                strip.transpose(2, 0, 1)).astype(ml_dtypes.bfloat16)
            # cross-attn exp(bias): [77, NH*512]
            cidx = 512 * c + qq[None, :] - kk[:, None] + (LC - 1)
            ebc = np.exp(ca_tab[l][cidx, :])                    # [77, 512, NH]
            m[f"ebca{l}"] = np.ascontiguousarray(
                ebc.transpose(0, 2, 1).reshape(LC, NH * TL)).astype(ml_dtypes.bfloat16)
        in_maps.append(m)
    return in_maps, any_bias


def build_program(any_bias=False, stage=6, layers=L):
    """Builds the Bacc/TileContext SPMD program (one NEFF, all 8 cores).

    stage: 1=LN1 only, 2=+CA, 3=+AG/xg, 4=+KVq prep, 5=+SA, 6=full
    """
    core_ids = list(range(N_CORES))
    nc = bacc.Bacc("TRN2", target_bir_lowering=False, debug=False,
                   num_devices=N_CORES)

    x0_d = nc.dram_tensor("x0", [2, 128, TL], F32, kind="ExternalInput")
    ctxT_d = nc.dram_tensor("ctxT", [6, 128, LC], BF16, kind="ExternalInput")
    cvec_d = nc.dram_tensor("cvec", [128, 4], F32, kind="ExternalInput")
    crow_d = nc.dram_tensor("crow", [1, 256], F32, kind="ExternalInput")
    wimg_d = [nc.dram_tensor(f"wimg{l}", [128, WIMG_COLS], BF16, kind="ExternalInput") for l in range(L)]
    bimg_d = [nc.dram_tensor(f"bimg{l}", [128, 20], F32, kind="ExternalInput") for l in range(L)]
    ebsa_d = [nc.dram_tensor(f"ebsa{l}", [NH, 128, STRIP_W], BF16, kind="ExternalInput") for l in range(L)]
    ebca_d = [nc.dram_tensor(f"ebca{l}", [LC, NH * TL], BF16, kind="ExternalInput") for l in range(L)]
    y_d = nc.dram_tensor("y", [2, 128, TL], F32, kind="ExternalOutput")

    HT = TL // 2
    ag_in_d = [[nc.dram_tensor(f"ag_in{l}_{a}", [2, 128, HT], BF16) for a in range(2)]
               for l in range(L)]
    ag_out_d = [[nc.dram_tensor(f"ag_out{l}_{a}", [N_CORES, 2, 128, HT], BF16,
                                addr_space="Shared") for a in range(2)] for l in range(L)]

    with tile.TileContext(nc) as tc:
        with (
            tc.tile_pool(name="pconst", bufs=1) as pconst,
            tc.tile_pool(name="ppers", bufs=1) as ppers,
            tc.tile_pool(name="px", bufs=2) as px,
            tc.tile_pool(name="pstrip", bufs=2) as pstrip,
            tc.tile_pool(name="pwork", bufs=3) as pwork,
            tc.tile_pool(name="prow", bufs=6) as prow,
            tc.tile_pool(name="pca_p", bufs=3) as pca_p,
            tc.tile_pool(name="psc", bufs=2, space="PSUM") as psc,
            tc.tile_pool(name="pacc", bufs=2, space="PSUM") as pacc,
            tc.tile_pool(name="ppp", bufs=2, space="PSUM") as ppp,
        ):
            # ---- constants / weights (x + layer-0 weights first; layer-1 deferred) ----
            x_t = []
            for ci in range(2):
                xt = px.tile([128, TL], F32, tag=f"x{ci}")
                nc.sync.dma_start(out=xt[:], in_=x0_d[ci])
                x_t.append(xt)
            cvec_t = pconst.tile([128, 4], F32, tag="cvec")
            crow_t = pconst.tile([1, 256], F32, tag="crow")
            ctx_t = pconst.tile([128, 6 * LC], BF16, tag="ctx")
            nc.sync.dma_start(out=cvec_t[:], in_=cvec_d[:])
            nc.sync.dma_start(out=crow_t[:], in_=crow_d[:])
            nc.sync.dma_start(
                out=ctx_t[:].rearrange("p (c t) -> p c t", c=6),
                in_=ctxT_d[:].rearrange("c p t -> p c t"))
            wimg_t, bimg_t = [], []
            for l in range(L):
                wt = pconst.tile([128, WIMG_COLS], BF16, tag=f"wimg{l}", name="wt")
                wimg_t.append(wt)
                bt = pconst.tile([128, 20], F32, tag=f"bimg{l}", name="bt")
                bimg_t.append(bt)
            nc.sync.dma_start(out=wimg_t[0][:], in_=wimg_d[0][:])
            nc.sync.dma_start(out=bimg_t[0][:], in_=bimg_d[0][:])

            ones_col = cvec_t[:, 0:1]
            ones_row = crow_t[0:1, 0:128]

            def w_ap(l, base, i, j, n_j, width=128):
                off = base + (i * n_j + j) * width
                return wimg_t[l][:, off:off + width]

            def layernorm(xin):
                ps_sum = ppp.tile([1, TL], F32, tag="pp")
                nc.tensor.matmul(ps_sum[:], ones_col, xin[0][:], start=True, stop=False)
                nc.tensor.matmul(ps_sum[:], ones_col, xin[1][:], start=False, stop=True)
                sq = []
                for ci in range(2):
                    st = pwork.tile([128, TL], F32, tag="sq")
                    nc.vector.tensor_mul(st[:], xin[ci][:], xin[ci][:])
                    sq.append(st)
                ps_sq = ppp.tile([1, TL], F32, tag="pp")
                nc.tensor.matmul(ps_sq[:], ones_col, sq[0][:], start=True, stop=False)
                nc.tensor.matmul(ps_sq[:], ones_col, sq[1][:], start=False, stop=True)
                m_sb = prow.tile([1, TL], F32, tag="row")
                nc.scalar.activation(m_sb[:], ps_sum[:], AF.Copy, scale=1.0 / C)
                m2_sb = prow.tile([1, TL], F32, tag="row")
                nc.scalar.activation(m2_sb[:], ps_sum[:], AF.Square, scale=1.0 / C)
                var_sb = prow.tile([1, TL], F32, tag="row")
                nc.vector.scalar_tensor_tensor(var_sb[:], ps_sq[:], 1.0 / C, m2_sb[:],
                                               op0=ALU.mult, op1=ALU.subtract)
                lv_sb = prow.tile([1, TL], F32, tag="row")
                nc.scalar.activation(lv_sb[:], var_sb[:], AF.Ln, bias=cvec_t[0:1, 2:3])
                rstd_sb = prow.tile([1, TL], F32, tag="row")
                nc.scalar.activation(rstd_sb[:], lv_sb[:], AF.Exp, scale=-0.5)
                ps_mbc = ppp.tile([128, TL], F32, tag="pp")
                nc.tensor.matmul(ps_mbc[:], ones_row, m_sb[:], start=True, stop=True)
                ps_rbc = ppp.tile([128, TL], F32, tag="pp")
                nc.tensor.matmul(ps_rbc[:], ones_row, rstd_sb[:], start=True, stop=True)
                out = []
                for ci in range(2):
                    tt = pwork.tile([128, TL], F32, tag="lnt")
                    nc.vector.tensor_sub(tt[:], xin[ci][:], ps_mbc[:])
                    ot = pwork.tile([128, TL], BF16, tag=f"xs{ci}")
                    nc.vector.tensor_mul(ot[:], tt[:], ps_rbc[:])
                    out.append(ot)
                return out

            def norm_head_out(l, o_ps, o_norm, h, bias_col):
                """o_ps: psum [33, TL] (row0 = sums); writes o_norm chunk rows."""
                lr = prow.tile([1, TL], F32, tag="row")
                nc.scalar.activation(lr[:], o_ps[32:33, :], AF.Ln)
                rr = prow.tile([1, TL], F32, tag="row")
                nc.scalar.activation(rr[:], lr[:], AF.Exp, scale=-1.0)
                rb_ps = ppp.tile([32, TL], F32, tag="pp")
                nc.tensor.matmul(rb_ps[:], crow_t[0:1, 0:32], rr[:], start=True, stop=True)
                rb_sb = pwork.tile([32, TL], F32, tag="rb")
                nc.vector.tensor_copy(rb_sb[:], rb_ps[:])
                r0 = 32 * (h % 4)
                dst = o_norm[h // 4][r0:r0 + 32, :]
                nc.vector.tensor_mul(dst, o_ps[0:32, :], rb_sb[:])
                if any_bias and bias_col is not None:
                    nc.vector.tensor_scalar_add(
                        dst, dst, bimg_t[l][r0:r0 + 32, bias_col + h // 4:bias_col + h // 4 + 1])

            def proj_residual(l, base_off, o_norm, bo_col):
                newx = []
                for oc in range(2):
                    ps = ppp.tile([128, TL], F32, tag="pp")
                    for ic in range(2):
                        nc.tensor.matmul(ps[:], w_ap(l, base_off, ic, oc, 2),
                                         o_norm[ic][:], start=(ic == 0), stop=(ic == 1))
                    nx = px.tile([128, TL], F32, tag=f"x{oc}")
                    nc.vector.scalar_tensor_tensor(
                        nx[:], ps[:], bimg_t[l][:, bo_col + oc:bo_col + oc + 1],
                        x_t[oc][:], op0=ALU.add, op1=ALU.add)
                    newx.append(nx)
                return newx

            for l in range(layers):
                # ================= cross-attention =================
                xs1 = layernorm(x_t)
                if stage < 2:
                    x_t = [xs1[0], xs1[1]]  # dummy: emit LN result
                    break
                ebca_t = pstrip.tile([LC, NH * TL], BF16, tag="ebca", bufs=1)
                nc.sync.dma_start(out=ebca_t[:], in_=ebca_d[l][:])
                q_ca = []
                for oc in range(2):
                    ps = ppp.tile([128, TL], F32, tag="pp")
                    for ic in range(2):
                        nc.tensor.matmul(ps[:], w_ap(l, OFF_CAQ, ic, oc, 2),
                                         xs1[ic][:], start=(ic == 0), stop=(ic == 1))
                    qt = ppers.tile([128, TL], BF16, tag=f"qca{oc}")
                    if any_bias:
                        nc.vector.tensor_scalar_add(qt[:], ps[:], bimg_t[l][:, oc:oc + 1])
                    else:
                        nc.vector.tensor_copy(qt[:], ps[:])
                    q_ca.append(qt)
                kca = []
                for oc in range(2):
                    ps = ppp.tile([128, LC], F32, tag="pp")
                    for ic in range(6):
                        nc.tensor.matmul(ps[:], w_ap(l, OFF_CAK, ic, oc, 2),
                                         ctx_t[:, ic * LC:(ic + 1) * LC],
                                         start=(ic == 0), stop=(ic == 5))
                    kt = ppers.tile([128, LC], BF16, tag=f"kca{oc}")
                    nc.vector.tensor_copy(kt[:], ps[:])
                    kca.append(kt)
                ps_v = ppp.tile([LC, 256], F32, tag="pp")
                for ic in range(6):
                    nc.tensor.matmul(
                        ps_v[:], ctx_t[:, ic * LC:(ic + 1) * LC],
                        wimg_t[l][:, OFF_CAV + ic * 256:OFF_CAV + (ic + 1) * 256],
                        start=(ic == 0), stop=(ic == 5))
                vca = ppers.tile([LC, NH * 33], BF16, tag="vca")
                vca_r = vca[:].rearrange("p (h c) -> p h c", h=NH)
                nc.vector.memset(vca_r[:, :, 32:33], 1.0)
                nc.vector.tensor_copy(
                    vca_r[:, :, 0:32], ps_v[:].rearrange("p (h c) -> p h c", h=NH))
                o_norm = [ppers.tile([128, TL], BF16, tag=f"onorm{ci}", name=f"onorm{ci}") for ci in range(2)]
                for hp in range(NH // 2):          # head pairs through the wide sc tiles
                    ps_s = psc.tile([128, 2 * TL], F32, tag="sc", name="ps_s")
                    for b in range(2):
                        h = 2 * hp + b
                        r0 = 32 * (h % 4)
                        k_ap = kca[h // 4][r0:r0 + 32, :]
                        q_ap = q_ca[h // 4][r0:r0 + 32, :]
                        if h % 4 == 3:  # PE can't read base partition 96 (quadrant-3 bug)
                            ks = pwork.tile([32, LC], BF16, tag="kscr_ca", name="ks", bufs=1)
                            nc.vector.tensor_copy(ks[:], k_ap)
                            qs = pwork.tile([32, TL], BF16, tag="qscr_ca", name="qs", bufs=1)
                            nc.vector.tensor_copy(qs[:], q_ap)
                            k_ap, q_ap = ks[:], qs[:]
                        nc.tensor.matmul(ps_s[0:LC, TL * b:TL * (b + 1)], k_ap, q_ap,
                                         start=True, stop=True)
                    pc = pca_p.tile([LC, 2 * TL], BF16, tag="pca")
                    nc.scalar.activation(pc[:], ps_s[0:LC, :], AF.Exp, scale=SCALE)
                    pc2 = pca_p.tile([LC, 2 * TL], BF16, tag="pca2")
                    nc.vector.tensor_mul(pc2[:], pc[:],
                                         ebca_t[:, 2 * hp * TL:2 * (hp + 1) * TL])
                    for b in range(2):
                        h = 2 * hp + b
                        o_ps = pacc.tile([33, TL], F32, tag="acc")
                        nc.tensor.matmul(o_ps[:], vca[:, h * 33:(h + 1) * 33],
                                         pc2[:, TL * b:TL * (b + 1)],
                                         start=True, stop=True)
                        norm_head_out(l, o_ps, o_norm, h, None)
                x_t = proj_residual(l, OFF_CAO, o_norm, 2)
                if stage < 3:
                    break

                # ================= self-attention =================
                xs2 = layernorm(x_t)
                for a in range(2):
                    for ci in range(2):
                        nc.sync.dma_start(out=ag_in_d[l][a][ci],
                                          in_=xs2[ci][:, HT * a:HT * (a + 1)])
                    nc.gpsimd.collective_compute(
                        "AllGather", ALU.bypass,
                        replica_groups=[core_ids],
                        ins=[ag_in_d[l][a][:]],
                        outs=[ag_out_d[l][a][:]],
                    )
                if l == 0:
                    nc.sync.dma_start(out=wimg_t[1][:], in_=wimg_d[1][:])
                    nc.sync.dma_start(out=bimg_t[1][:], in_=bimg_d[1][:])
                # xg halves: xg[a][ci] is [128, N_CORES*HT], col s*HT+t = token 512s+256a+t
                xg = [[], []]
                for a in range(2):
                    for ci in range(2):
                        xgt = ppers.tile([128, N_CORES * HT], BF16,
                                         tag=f"xg{ci}_{a}", name="xgt")
                        nc.sync.dma_start(
                            out=xgt[:].rearrange("p (s t) -> p s t", s=N_CORES),
                            in_=ag_out_d[l][a][:, ci].rearrange("s p t -> p s t"))
                        xg[a].append(xgt)
                if stage < 4:
                    x_t = [xs2[0], xs2[1]]
                    break
                qT = []
                for oc in range(2):
                    ps = ppp.tile([128, TL], F32, tag="pp")
                    for ic in range(2):
                        nc.tensor.matmul(ps[:], w_ap(l, OFF_SAQ, ic, oc, 2),
                                         xs2[ic][:], start=(ic == 0), stop=(ic == 1))
                    qt = ppers.tile([128, TL], BF16, tag=f"qsa{oc}")
                    if any_bias:
                        nc.vector.tensor_scalar_add(qt[:], ps[:], bimg_t[l][:, 4 + oc:5 + oc])
                    else:
                        nc.vector.tensor_copy(qt[:], ps[:])
                    qT.append(qt)
                kT = [ppers.tile([128, S], BF16, tag=f"ksa{oc}", name="kt")
                      for oc in range(2)]
                vaug = ppers.tile([128, NKT * 264], BF16, tag="vaug")
                vaug_r = vaug[:].rearrange("p (k h c) -> p k h c", k=NKT, h=NH)
                nc.vector.memset(vaug_r[:, :, :, 32:33], 1.0)
                for a in range(2):          # half A fully before half B
                    for oc in range(2):
                        for w in range(8):
                            ps = ppp.tile([128, HT], F32, tag="pp")
                            for ic in range(2):
                                nc.tensor.matmul(ps[:], w_ap(l, OFF_SAK, ic, oc, 2),
                                                 xg[a][ic][:, HT * w:HT * (w + 1)],
                                                 start=(ic == 0), stop=(ic == 1))
                            nc.vector.tensor_copy(
                                kT[oc][:, 512 * w + HT * a:512 * w + HT * (a + 1)], ps[:])
                    for kt_i in range(NKT):
                        if (kt_i % 4) // 2 != a:
                            continue
                        src_off = HT * (kt_i // 4) + 128 * (kt_i % 2)
                        ps = ppp.tile([128, 256], F32, tag="pp")
                        for ic in range(2):
                            nc.tensor.matmul(
                                ps[:], xg[a][ic][:, src_off:src_off + 128],
                                wimg_t[l][:, OFF_SAV + ic * 256:OFF_SAV + (ic + 1) * 256],
                                start=(ic == 0), stop=(ic == 1))
                        nc.vector.tensor_copy(
                            vaug_r[:, kt_i, :, 0:32],
                            ps[:].rearrange("p (h c) -> p h c", h=NH))
                if stage < 5:
                    break
                o_norm = [ppers.tile([128, TL], BF16, tag=f"onorm{ci}", name=f"onorm{ci}") for ci in range(2)]
                for h in range(NH):
                    strip = pstrip.tile([128, STRIP_W], BF16, tag="strip")
                    nc.sync.dma_start(out=strip[:], in_=ebsa_d[l][h])
                    o_ps = pacc.tile([33, TL], F32, tag="acc")
                    r0 = 32 * (h % 4)
                    if h % 4 == 3:  # PE can't read base partition 96 (quadrant-3 bug)
                        khs = pwork.tile([32, S], BF16, tag="kscr_sa", name="khs", bufs=1)
                        nc.vector.tensor_copy(khs[:], kT[h // 4][r0:r0 + 32, :])
                        qhs = pwork.tile([32, TL], BF16, tag="qscr_sa", name="qhs", bufs=1)
                        nc.vector.tensor_copy(qhs[:], qT[h // 4][r0:r0 + 32, :])
                        kh, qh, r0 = khs, qhs[:], 0
                    else:
                        kh = kT[h // 4]
                        qh = qT[h // 4][r0:r0 + 32, :]
                    g_order = [g for g in range(NG) if g % 2 == 0] + \
                              [g for g in range(NG) if g % 2 == 1]
                    first_t, last_t = g_order[0] * GK, g_order[-1] * GK + GK - 1
                    for g in g_order:
                        ps_s = psc.tile([128, GK * TL], F32, tag="sc")
                        for j in range(GK):
                            t = g * GK + j
                            nc.tensor.matmul(
                                ps_s[:, TL * j:TL * (j + 1)],
                                kh[r0:r0 + 32, 128 * t:128 * (t + 1)],
                                qh, start=True, stop=True)
                        pt = pwork.tile([128, GK * TL], BF16, tag="pt")
                        nc.scalar.activation(pt[:], ps_s[:], AF.Exp, scale=SCALE)
                        p2 = pwork.tile([128, GK * TL], BF16, tag="p2")
                        for j in range(GK):
                            t = g * GK + j
                            off = 3968 - 128 * t
                            nc.vector.tensor_mul(
                                p2[:, TL * j:TL * (j + 1)],
                                pt[:, TL * j:TL * (j + 1)],
                                strip[:, off:off + TL])
                        for j in range(GK):
                            t = g * GK + j
                            nc.tensor.matmul(
                                o_ps[:], vaug[:, 264 * t + 33 * h:264 * t + 33 * (h + 1)],
                                p2[:, TL * j:TL * (j + 1)],
                                start=(t == first_t), stop=(t == last_t))
                    norm_head_out(l, o_ps, o_norm, h, 6)
                x_t = proj_residual(l, OFF_SAO, o_norm, 8)
                if stage < 6:
                    break

                # ================= FFN (replaces x) =================
                xs3 = layernorm(x_t)
                ps_f = [ppp.tile([128, TL], F32, tag="pp", name="ps_f") for _ in range(2)]
                for hp in range(4):                 # hidden-chunk pairs via sc tiles
                    ps_h = psc.tile([128, 2 * TL], F32, tag="sc", name="ps_h")
                    for b in range(2):
                        hc = 2 * hp + b
                        for ic in range(2):
                            nc.tensor.matmul(ps_h[:, TL * b:TL * (b + 1)],
                                             w_ap(l, OFF_W1, ic, hc, 8),
                                             xs3[ic][:], start=(ic == 0), stop=(ic == 1))
                    gt = pwork.tile([128, 2 * TL], BF16, tag="gelu", name="gt")
                    if any_bias:
                        for b in range(2):
                            nc.scalar.activation(
                                gt[:, TL * b:TL * (b + 1)], ps_h[:, TL * b:TL * (b + 1)],
                                AF.Gelu, bias=bimg_t[l][:, 10 + 2 * hp + b:11 + 2 * hp + b])
                    else:
                        nc.scalar.activation(gt[:], ps_h[:], AF.Gelu)
                    for b in range(2):
                        hc = 2 * hp + b
                        for oc in range(2):
                            nc.tensor.matmul(ps_f[oc][:], w_ap(l, OFF_W2, hc, oc, 2),
                                             gt[:, TL * b:TL * (b + 1)],
                                             start=(hc == 0), stop=(hc == 7))
                newx = []
                for oc in range(2):
                    nx = px.tile([128, TL], F32, tag=f"x{oc}")
                    nc.vector.tensor_scalar_add(nx[:], ps_f[oc][:],
                                                bimg_t[l][:, 18 + oc:19 + oc])
                    newx.append(nx)
                x_t = newx

            for ci in range(2):
                if x_t[ci].dtype != F32:
                    cast = ppers.tile([128, TL], F32, tag=f"cast{ci}", name="cast")
                    nc.vector.tensor_copy(cast[:], x_t[ci][:])
                    nc.sync.dma_start(out=y_d[ci], in_=cast[:])
                else:
                    nc.sync.dma_start(out=y_d[ci], in_=x_t[ci][:])

    nc.compile()
    return nc


_CACHE = {}


def _get_program(any_bias):
    key = bool(any_bias)
    if key not in _CACHE:
        _CACHE[key] = build_program(any_bias=key)
    return _CACHE[key]


def kernel(**inputs) -> np.ndarray:
    in_maps, any_bias = build_in_maps(inputs)
    nc = _get_program(any_bias)
    res = run_bass_kernel_spmd(nc, in_maps, list(range(N_CORES)))
    slices = [np.asarray(res.results[c]["y"], np.float32).reshape(C, TL)
              for c in range(N_CORES)]
    full = np.concatenate(slices, axis=1)                      # [C, S]
    return full.reshape(B, C, W_SP, H_SP).astype(np.float32)
